# revision 1
# baseline (speedup 1.0000x reference)
"""Binarized LeNet5 + BN (CIM-style) forward on 8 Trainium2 NeuronCores.

Pure data parallel: batch 8192 -> 1024 images/core, processed in 8 blocks of
128 images, fully on-chip per block, two-stage software pipeline across blocks.

Per-block stages (per core):
  A: DMA x-block -> XF [28,(img,28)] f32; PE broadcast SEL1 [28->120=(j,dy)];
     DVE is_ge -> XR {0,1} bf16; conv1 (2 o-chunks x 5 dx matmuls, K=120,
     M=72=(jp,j2,o)); fused BN+clip as t=min(max(z+c,lo),hi) on DVE; poolx
     DVE; pooly via signed pool-matrix matmul on PE; DVE is_ge -> H1u.
  B: PE broadcast SEL2_g [36x2->120=(j,c,dy)] -> XR2 {0,1}; conv2 (per g,
     5 dx matmuls K=120, M=64); same fused post; fc realign via selector
     matmuls; fc1/fc2 + ACT Sign; fc3 + ACT Identity; PE transpose; DMA out.

All weights/BN constants are folded host-side in numpy and passed as inputs.
The BN fold: y=clip(a*z+b,-1,1) with a=2*inv (step inputs) pools to
sign(sum y) = sign(sgn(a) * sum clip(z + b/a, -1/|a|, 1/|a|)); sgn(a) lives
in the pool matrix entries.
"""
import sys

sys.path.insert(0, "/opt/trn_rl_repo")

import numpy as np

NCORES = 8
B = 8192
BPC = B // NCORES          # images per core
NB = 128                   # images per block
NBLK = BPC // NB           # blocks per core
EPS = 1e-5

_cache = {}


def _binz(w):
    return np.where(w >= 0, 1.0, -1.0).astype(np.float32)


def build_constants(d):
    C = {}
    # ---- conv1 ({0,1} input): h = 2*M - rowsum + b ----
    w1 = _binz(d['conv1_w'])
    lhsT1 = np.zeros((2, 5, 120, 72), np.float32)
    for chunk in range(2):
        for dx in range(5):
            for j in range(24):
                for dy in range(5):
                    p = j * 5 + dy
                    jp, j2 = j % 2, j // 2
                    m = jp * 36 + j2 * 3 + np.arange(3)
                    lhsT1[chunk, dx, p, m] = w1[chunk * 3 + np.arange(3), 0, dy, dx]
    C['lhsT1'] = lhsT1
    SEL1 = np.zeros((28, 120), np.float32)
    for j in range(24):
        for dy in range(5):
            SEL1[j + dy, j * 5 + dy] = 1.0
    C['SEL1'] = SEL1
    inv1 = d['bn1_g'] / np.sqrt(d['bn1_v'] + EPS)
    a1 = inv1          # conv1 input is +-1 (ACT Sign)
    b1 = inv1 * d['conv1_b'] + d['bn1_b'] - d['bn1_m'] * inv1
    c1 = b1 / a1
    lo1 = -1.0 / np.abs(a1)
    hi1 = 1.0 / np.abs(a1)
    sg1 = np.where(a1 >= 0, 1.0, -1.0).astype(np.float32)

    def expand72(vec6, chunk):
        out = np.zeros(72, np.float32)
        for jp in range(2):
            for j2 in range(12):
                out[jp * 36 + j2 * 3 + np.arange(3)] = vec6[chunk * 3 + np.arange(3)]
        return out

    L1 = (lo1 - c1).astype(np.float32)
    H1 = (hi1 - c1).astype(np.float32)
    # tie at full saturation must be exact: threshold = 2*(L+H)*sg in f32
    tau1 = (np.float32(2.0) * (L1 + H1) * sg1).astype(np.float32)
    C['c1'] = np.stack([expand72(c1, c) for c in range(2)])
    C['L1'] = np.stack([expand72(L1, c) for c in range(2)])
    C['H1'] = np.stack([expand72(H1, c) for c in range(2)])
    C['tau1'] = np.stack([expand72(tau1, c) for c in range(2)])
    C['sg1'] = np.stack([expand72(sg1, c) for c in range(2)])  # for poolm entries

    # ---- conv2 ({0,1} input) ----
    w2 = _binz(d['conv2_w'])
    SEL2 = np.zeros((2, 72, 120), np.float32)
    for g in range(2):
        for chunk in range(2):
            for j2 in range(12):
                for ol in range(3):
                    ps = chunk * 36 + j2 * 3 + ol
                    c = chunk * 3 + ol
                    for j in range(4):
                        for dy in range(5):
                            if j2 == 4 * g + j + dy:
                                SEL2[g, ps, j * 30 + c * 5 + dy] = 1.0
    C['SEL2'] = SEL2
    lhsT2 = np.zeros((5, 120, 64), np.float32)
    for dx in range(5):
        for j in range(4):
            for c in range(6):
                for dy in range(5):
                    p2 = j * 30 + c * 5 + dy
                    jp, j2 = j % 2, j // 2
                    lhsT2[dx, p2, jp * 32 + j2 * 16 + np.arange(16)] = w2[:, c, dy, dx]
    C['lhsT2'] = lhsT2
    inv2 = d['bn2_g'] / np.sqrt(d['bn2_v'] + EPS)
    rs2 = w2.reshape(16, -1).sum(1)
    a2 = 2.0 * inv2
    b2 = inv2 * (d['conv2_b'] - rs2) + d['bn2_b'] - d['bn2_m'] * inv2
    c2 = b2 / a2
    lo2 = -1.0 / np.abs(a2)
    hi2 = 1.0 / np.abs(a2)
    sg2 = np.where(a2 >= 0, 1.0, -1.0).astype(np.float32)

    def expand64(vec16):
        out = np.zeros(64, np.float32)
        for jp in range(2):
            for j2 in range(2):
                out[jp * 32 + j2 * 16 + np.arange(16)] = vec16
        return out

    L2 = (lo2 - c2).astype(np.float32)
    H2 = (hi2 - c2).astype(np.float32)
    tau2 = (np.float32(2.0) * (L2 + H2) * sg2).astype(np.float32)
    C['c2'] = expand64(c2)
    C['L2'] = expand64(L2)
    C['H2'] = expand64(H2)
    C['tau2'] = expand64(tau2)
    C['sg2'] = expand64(sg2)

    # ---- fc1 ({0,1} input) ----
    wf1 = _binz(d['fc1_w'])
    lf1 = np.zeros((2, 128, 120), np.float32)
    for t in range(2):
        for xl in range(2):
            xp = t * 2 + xl
            for g in range(2):
                for j2 in range(2):
                    y2 = 2 * g + j2
                    for o2 in range(16):
                        f = xl * 64 + g * 32 + j2 * 16 + o2
                        k = o2 * 16 + y2 * 4 + xp
                        lf1[t, f] = wf1[:, k]
    C['lhsTf1'] = lf1
    invf1 = d['bn_fc1_g'] / np.sqrt(d['bn_fc1_v'] + EPS)
    rsf1 = wf1.sum(1)
    C['bnf1_scale'] = (2.0 * invf1).astype(np.float32)
    C['bnf1_shift'] = (invf1 * (d['fc1_b'] - rsf1) + d['bn_fc1_b']
                      - d['bn_fc1_m'] * invf1).astype(np.float32)
    # ---- fc2 (+-1 input) ----
    wf2 = _binz(d['fc2_w'])
    C['lhsTf2'] = np.ascontiguousarray(wf2.T)
    invf2 = d['bn_fc2_g'] / np.sqrt(d['bn_fc2_v'] + EPS)
    C['bnf2_scale'] = invf2.astype(np.float32)
    C['bnf2_shift'] = (invf2 * d['fc2_b'] + d['bn_fc2_b']
                      - d['bn_fc2_m'] * invf2).astype(np.float32)
    # ---- fc3 (+-1 input) ----
    wf3 = _binz(d['fc3_w'])
    C['lhsTf3'] = np.ascontiguousarray(wf3.T)
    invf3 = d['bn_fc3_g'] / np.sqrt(d['bn_fc3_v'] + EPS)
    C['bnf3_scale'] = invf3.astype(np.float32)
    C['bnf3_shift'] = (invf3 * d['fc3_b'] + d['bn_fc3_b']
                      - d['bn_fc3_m'] * invf3).astype(np.float32)
    return C


def pack_constants(C):
    t = {}
    t['sel1'] = C['SEL1']                                          # [28,120] f32
    l1 = np.zeros((120, 720), np.float32)
    for chunk in range(2):
        for dx in range(5):
            l1[:, (chunk * 5 + dx) * 72:(chunk * 5 + dx + 1) * 72] = C['lhsT1'][chunk, dx]
    t['lhsT1'] = l1
    s2 = np.zeros((36, 480), np.float32)
    for g in range(2):
        for chunk in range(2):
            s2[:, (g * 2 + chunk) * 120:(g * 2 + chunk + 1) * 120] = \
                C['SEL2'][g][chunk * 36:(chunk + 1) * 36]
    t['sel2'] = s2
    # signed pool matrices: conv1 per chunk (sg differs by chunk)
    pm1 = np.zeros((72, 72), np.float32)   # cols: [chunk0 36 | chunk1 36]
    for chunk in range(2):
        for m in range(36):
            pm1[m, chunk * 36 + m] = C['sg1'][chunk][m]
            pm1[m + 36, chunk * 36 + m] = C['sg1'][chunk][m]
    t['poolm1'] = pm1
    pm2 = np.zeros((64, 32), np.float32)
    for m in range(32):
        pm2[m, m] = C['sg2'][m]
        pm2[m + 32, m] = C['sg2'][m]
    t['poolm2'] = pm2
    sf = np.zeros((32, 8 * 128), np.float32)
    for tt in range(2):
        for xl in range(2):
            for g in range(2):
                idx = (tt * 2 + xl) * 2 + g
                for s in range(32):
                    sf[s, idx * 128 + xl * 64 + g * 32 + s] = 1.0
    t['selfm'] = sf
    l2 = np.zeros((120, 320), np.float32)
    for dx in range(5):
        l2[:, dx * 64:(dx + 1) * 64] = C['lhsT2'][dx]
    t['lhsT2'] = l2
    lf = np.zeros((128, 240), np.float32)
    lf[:, :120] = C['lhsTf1'][0]
    lf[:, 120:] = C['lhsTf1'][1]
    t['lhsTf1'] = lf
    t['lhsTf2'] = C['lhsTf2']
    t['lhsTf3'] = C['lhsTf3']
    # bn vectors packed [128, 16] f32
    bn = np.zeros((128, 16), np.float32)
    bn[:72, 0] = C['L1'][0]; bn[:72, 1] = C['H1'][0]; bn[:72, 2] = C['tau1'][0]
    bn[:72, 3] = C['L1'][1]; bn[:72, 4] = C['H1'][1]; bn[:72, 5] = C['tau1'][1]
    bn[:64, 6] = C['L2'];    bn[:64, 7] = C['H2'];    bn[:64, 8] = C['tau2']
    bn[:120, 9] = C['bnf1_scale'];  bn[:120, 10] = C['bnf1_shift']
    bn[:84, 11] = C['bnf2_scale'];  bn[:84, 12] = C['bnf2_shift']
    t['bnv'] = bn
    bn3 = np.zeros((10, 2), np.float32)
    bn3[:, 0] = C['bnf3_scale']; bn3[:, 1] = C['bnf3_shift']
    t['bnv3'] = bn3
    t['ident10'] = np.eye(10, dtype=np.float32)
    return t


def legalize_waits(nc, max_waits=1):
    """Split multi-wait instructions: this walrus supports one sync wait per
    instruction, so move excess waits onto preceding same-engine NoOps."""
    import concourse.mybir as mybir
    n_split = 0
    for f in nc.m.functions:
        for b in f.blocks:
            new = []
            for inst in b.instructions:
                si = inst.sync_info
                if si is not None and len(si.on_wait) > max_waits:
                    waits = list(si.on_wait)
                    excess, keep = waits[:-max_waits], waits[-max_waits:]
                    for k, w in enumerate(excess):
                        nop = mybir.InstNoOp(name=f'{inst.name}-w{k}', ins=[], outs=[])
                        nop.engine = inst.engine
                        nop.sync_info = mybir.SyncInfo(on_wait=[w], on_update=[])
                        new.append(nop)
                    inst.sync_info = mybir.SyncInfo(on_wait=keep,
                                                    on_update=list(si.on_update))
                    n_split += 1
                new.append(inst)
            b.instructions[:] = new
    return n_split


def build_bass(debug_dump=False):
    import concourse.bass as bass
    import concourse.mybir as mybir
    from concourse.tile import TileContext

    f32 = mybir.dt.float32
    bf16 = mybir.dt.bfloat16
    AF = mybir.ActivationFunctionType
    OP = mybir.AluOpType

    nc = bass.Bass("TRN2", target_bir_lowering=False, debug=False,
                   enable_asserts=False, num_devices=NCORES)

    x_in = nc.dram_tensor("x", [BPC, 784], f32, kind="ExternalInput")
    sel1_in = nc.dram_tensor("sel1", [28, 120], f32, kind="ExternalInput")
    lhsT1_in = nc.dram_tensor("lhsT1", [120, 720], f32, kind="ExternalInput")
    sel2_in = nc.dram_tensor("sel2", [36, 480], f32, kind="ExternalInput")
    poolm1_in = nc.dram_tensor("poolm1", [72, 72], f32, kind="ExternalInput")
    poolm2_in = nc.dram_tensor("poolm2", [64, 32], f32, kind="ExternalInput")
    selfm_in = nc.dram_tensor("selfm", [32, 1024], f32, kind="ExternalInput")
    lhsT2_in = nc.dram_tensor("lhsT2", [120, 320], f32, kind="ExternalInput")
    lhsTf1_in = nc.dram_tensor("lhsTf1", [128, 240], f32, kind="ExternalInput")
    lhsTf2_in = nc.dram_tensor("lhsTf2", [120, 84], f32, kind="ExternalInput")
    lhsTf3_in = nc.dram_tensor("lhsTf3", [84, 10], f32, kind="ExternalInput")
    bnv_in = nc.dram_tensor("bnv", [128, 16], f32, kind="ExternalInput")
    bnv3_in = nc.dram_tensor("bnv3", [10, 2], f32, kind="ExternalInput")
    id10_in = nc.dram_tensor("ident10", [10, 10], f32, kind="ExternalInput")
    out_t = nc.dram_tensor("out", [BPC, 10], f32, kind="ExternalOutput")
    dbg = {}
    if debug_dump:
        for nm, shp, dt in [("d_xr", [120, NB * 28], bf16),
                            ("d_h1u0", [36, NB * 12], bf16),
                            ("d_h1u1", [36, NB * 12], bf16),
                            ("d_h2u0", [32, NB * 4], bf16),
                            ("d_h2u1", [32, NB * 4], bf16),
                            ("d_f1", [120, NB], bf16), ("d_of", [10, NB], f32)]:
            dbg[nm] = nc.dram_tensor(nm, shp, dt, kind="ExternalOutput")

    NI = 16               # conv1 images per matmul (16*24 = 384 cols)
    NI2 = 32              # conv2 images per matmul (32*8 = 256 cols)

    with TileContext(nc) as tc:
        with (
            tc.tile_pool(name="consts", bufs=1) as cpool,
            tc.tile_pool(name="xf", bufs=2) as xfpool,
            tc.tile_pool(name="xr", bufs=2) as xrpool,
            tc.tile_pool(name="work", bufs=3) as wpool,
            tc.tile_pool(name="h1u", bufs=2) as h1pool,
            tc.tile_pool(name="xr2", bufs=2) as xr2pool,
            tc.tile_pool(name="fc", bufs=2) as fcpool,
            tc.tile_pool(name="psA", bufs=2, space="PSUM") as psA,
            tc.tile_pool(name="psB", bufs=2, space="PSUM") as psB,
            tc.tile_pool(name="psC", bufs=1, space="PSUM") as psC,
        ):
            def load_const(name, src_t, shape, dtype):
                if dtype == bf16:
                    tf = cpool.tile(shape, f32, tag=name + "f", name=name + "f")
                    nc.gpsimd.dma_start(out=tf[:, :], in_=src_t.ap())
                    tl = cpool.tile(shape, bf16, tag=name, name=name)
                    nc.vector.tensor_copy(tl[:, :], tf[:, :])
                    return tl
                tl = cpool.tile(shape, f32, tag=name, name=name)
                nc.gpsimd.dma_start(out=tl[:, :], in_=src_t.ap())
                return tl

            sel1 = load_const("sel1", sel1_in, [28, 120], f32)
            lhsT1 = load_const("lhsT1", lhsT1_in, [120, 720], bf16)
            sel2 = load_const("sel2", sel2_in, [36, 480], bf16)
            poolm1 = load_const("poolm1", poolm1_in, [72, 72], f32)
            poolm2 = load_const("poolm2", poolm2_in, [64, 32], f32)
            selfm = load_const("selfm", selfm_in, [32, 1024], bf16)
            lhsT2 = load_const("lhsT2", lhsT2_in, [120, 320], bf16)
            lhsTf1 = load_const("lhsTf1", lhsTf1_in, [128, 240], bf16)
            lhsTf2 = load_const("lhsTf2", lhsTf2_in, [120, 84], bf16)
            lhsTf3 = load_const("lhsTf3", lhsTf3_in, [84, 10], bf16)
            bnv = load_const("bnv", bnv_in, [128, 16], f32)
            bnv3 = load_const("bnv3", bnv3_in, [10, 2], f32)
            id10 = load_const("id10", id10_in, [10, 10], f32)

            def stage_a(blk):
                """DMA + bcast1 + binarize + conv1 + pool -> h1u tiles."""
                xf = xfpool.tile([28, NB * 28], f32, tag="xf", name="xf")
                src = x_in.ap()[blk * NB:(blk + 1) * NB, :].rearrange(
                    "i (r x) -> r i x", x=28)
                nc.gpsimd.dma_start(
                    out=xf[:, :].rearrange("r (i x) -> r i x", x=28), in_=src)

                xr = xrpool.tile([120, NB * 28], bf16, tag="xr", name="xr")
                for c0 in range(0, NB * 28, 448):
                    pb = psA.tile([120, 448], f32, tag="bcast", name="pb")
                    nc.tensor.matmul(pb[:, :], sel1[:, :], xf[:, c0:c0 + 448],
                                     start=True, stop=True)
                    nc.scalar.activation(xr[:, c0:c0 + 448], pb[:, :], AF.Sign)

                h1u_c = [h1pool.tile([36, NB * 12], bf16, tag=f"h1u{c}",
                                     name=f"h1u{c}") for c in range(2)]
                xrv = xr[:, :].rearrange("p (i x) -> p i x", x=28)
                for chunk in range(2):
                    svs = []
                    for pair in range(4):          # pairs of NI-image groups
                        ps = psB.tile([72, 1024], f32, tag="cps", name="c1ps")
                        for sub in range(2):
                            i0 = (pair * 2 + sub) * NI
                            for dx in range(5):
                                rhs = xrv[:, i0:i0 + NI, dx:dx + 24]
                                nc.tensor.matmul(
                                    ps[:, sub * 512:sub * 512 + NI * 24],
                                    lhsT1[:, (chunk * 5 + dx) * 72:(chunk * 5 + dx + 1) * 72],
                                    rhs, start=(dx == 0), stop=(dx == 4))
                        # q = clip2(z, L, H): one fused (max, min) op over both halves
                        s = wpool.tile([72, 2 * NI * 24], f32, tag="s1", name="s1")
                        psv = ps[:, :].rearrange("p (two n) -> p two n", two=2)
                        sv = s[:, :].rearrange("p (two n) -> p two n", two=2)
                        nc.vector.tensor_scalar(
                            sv[:, :, :NI * 24], psv[:, :, :NI * 24],
                            bnv[:72, 3 * chunk:3 * chunk + 1],
                            bnv[:72, 3 * chunk + 1:3 * chunk + 2],
                            op0=OP.max, op1=OP.min)
                        svs.append(s)
                    # pool x+y in one PE pass pair: 2 accumulating matmuls with
                    # stride-2 column APs; then threshold on DVE
                    for pair in range(4):
                        s = svs[pair]
                        sq = s[:, :].rearrange("p (sub i x2 xp) -> p sub i x2 xp",
                                               sub=2, i=NI, xp=2)
                        psy = psB.tile([36, 2 * NI * 12], f32, tag="cpy", name="pyps", bufs=1)
                        pyv = psy[:, :].rearrange("p (sub i x2) -> p sub i x2",
                                                  sub=2, i=NI)
                        for xp in range(2):
                            nc.tensor.matmul(pyv,
                                             poolm1[:, 36 * chunk:36 * (chunk + 1)],
                                             sq[:, :, :, :, xp],
                                             start=(xp == 0), stop=(xp == 1))
                        nc.vector.tensor_scalar(
                            h1u_c[chunk][:, pair * 384:(pair + 1) * 384],
                            psy[:, :], bnv[:36, 3 * chunk + 2:3 * chunk + 3],
                            None, op0=OP.is_ge)
                return xr, h1u_c


            def stage_b(blk, h1u_c):
                """bcast2 + conv2 + pool + fc + out."""
                xr2 = xr2pool.tile([120, 2 * NB * 12], bf16, tag="xr2", name="xr2")
                for g in range(2):
                    ncols2 = NB * 12
                    for c0 in range(0, ncols2, 384):
                        pb = psA.tile([120, 448], f32, tag="bcast", name="pb2")
                        for chunk in range(2):
                            nc.tensor.matmul(
                                pb[:, :384],
                                sel2[:, (g * 2 + chunk) * 120:(g * 2 + chunk + 1) * 120],
                                h1u_c[chunk][:, c0:c0 + 384],
                                start=(chunk == 0), stop=(chunk == 1))
                        nc.scalar.activation(
                            xr2[:, g * ncols2 + c0:g * ncols2 + c0 + 384],
                            pb[:, :384], AF.Copy)

                h2u_g = [fcpool.tile([32, NB * 4], bf16, tag=f"h2u{g}",
                                     name=f"h2u{g}") for g in range(2)]
                for g in range(2):
                    xr2v = xr2[:, g * NB * 12:(g + 1) * NB * 12].rearrange(
                        "p (i x) -> p i x", x=12)
                    svs = []
                    for pair in range(2):
                        ps = psB.tile([64, 1024], f32, tag="cps", name="c2ps")
                        for sub in range(2):
                            i0 = (pair * 2 + sub) * NI2
                            for dx in range(5):
                                rhs = xr2v[:, i0:i0 + NI2, dx:dx + 8]
                                nc.tensor.matmul(
                                    ps[:, sub * 512:sub * 512 + NI2 * 8],
                                    lhsT2[:, dx * 64:(dx + 1) * 64],
                                    rhs, start=(dx == 0), stop=(dx == 4))
                        s = wpool.tile([64, 2 * NI2 * 8], f32, tag="s2", name="s2")
                        psv = ps[:, :].rearrange("p (two n) -> p two n", two=2)
                        sv = s[:, :].rearrange("p (two n) -> p two n", two=2)
                        nc.vector.tensor_scalar(
                            sv[:, :, :NI2 * 8], psv[:, :, :NI2 * 8],
                            bnv[:64, 6:7], bnv[:64, 7:8], op0=OP.max, op1=OP.min)
                        svs.append(s)
                    for pair in range(2):
                        s = svs[pair]
                        sq = s[:, :].rearrange("p (sub i x2 xp) -> p sub i x2 xp",
                                               sub=2, i=NI2, xp=2)
                        psy = psB.tile([32, 2 * NI2 * 4], f32, tag="cpy", name="py2ps", bufs=1)
                        pyv = psy[:, :].rearrange("p (sub i x2) -> p sub i x2",
                                                  sub=2, i=NI2)
                        for xp in range(2):
                            nc.tensor.matmul(pyv, poolm2[:, :], sq[:, :, :, :, xp],
                                             start=(xp == 0), stop=(xp == 1))
                        nc.vector.tensor_scalar(
                            h2u_g[g][:, pair * 256:(pair + 1) * 256],
                            psy[:, :], bnv[:32, 8:9], None, op0=OP.is_ge)

                # FC stage
                rf = [fcpool.tile([128, NB], bf16, tag=f"rf{t}", name=f"rf{t}")
                      for t in range(2)]
                h2v = [h2u_g[g][:, :].rearrange("p (i x) -> p i x", x=4)
                       for g in range(2)]
                for t in range(2):
                    prf = psC.tile([128, NB], f32, tag="fcps", name="prf")
                    k = 0
                    for xl in range(2):
                        for g in range(2):
                            idx = (t * 2 + xl) * 2 + g
                            nc.tensor.matmul(
                                prf[:, :], selfm[:, idx * 128:(idx + 1) * 128],
                                h2v[g][:, :, t * 2 + xl],
                                start=(k == 0), stop=(k == 3))
                            k += 1
                    nc.scalar.activation(rf[t][:, :], prf[:, :], AF.Copy)
                psf = psC.tile([120, NB], f32, tag="fcps", name="psf")
                nc.tensor.matmul(psf[:, :], lhsTf1[:, 0:120], rf[0][:, :],
                                 start=True, stop=False)
                nc.tensor.matmul(psf[:, :], lhsTf1[:, 120:240], rf[1][:, :],
                                 start=False, stop=True)
                f1 = fcpool.tile([120, NB], bf16, tag="f1", name="f1")
                nc.scalar.activation(f1[:, :], psf[:, :], AF.Sign,
                                     bias=bnv[:120, 10:11], scale=bnv[:120, 9:10])
                psf2 = psC.tile([84, NB], f32, tag="fcps", name="psf2")
                nc.tensor.matmul(psf2[:, :], lhsTf2[:, :], f1[:, :],
                                 start=True, stop=True)
                f2 = fcpool.tile([84, NB], bf16, tag="f2", name="f2")
                nc.scalar.activation(f2[:, :], psf2[:, :], AF.Sign,
                                     bias=bnv[:84, 12:13], scale=bnv[:84, 11:12])
                psf3 = psC.tile([10, NB], f32, tag="fcps", name="psf3")
                nc.tensor.matmul(psf3[:, :], lhsTf3[:, :], f2[:, :],
                                 start=True, stop=True)
                of = fcpool.tile([10, NB], f32, tag="of", name="of")
                nc.scalar.activation(of[:, :], psf3[:, :], AF.Identity,
                                     bias=bnv3[:, 1:2], scale=bnv3[:, 0:1])
                pst = psC.tile([NB, 10], f32, tag="fcps", name="pst")
                nc.tensor.transpose(pst[:, :], of[:, :], id10[:, :])
                ot = fcpool.tile([NB, 10], f32, tag="ot", name="ot")
                nc.scalar.activation(ot[:, :], pst[:, :], AF.Copy)
                nc.sync.dma_start(out=out_t.ap()[blk * NB:(blk + 1) * NB, :],
                                  in_=ot[:, :])
                return h2u_g, f1, of

            # two-stage software pipeline over blocks
            nblk = 1 if debug_dump else NBLK
            pending = None
            for blk in range(nblk + 1):
                new_pending = None
                if blk < nblk:
                    xr, h1u_c = stage_a(blk)
                    if debug_dump:
                        nc.sync.dma_start(out=dbg["d_xr"].ap(), in_=xr[:, :])
                        nc.sync.dma_start(out=dbg["d_h1u0"].ap(), in_=h1u_c[0][:, :])
                        nc.sync.dma_start(out=dbg["d_h1u1"].ap(), in_=h1u_c[1][:, :])
                    new_pending = (blk, h1u_c)
                if pending is not None:
                    pb_blk, pb_h1u = pending
                    h2u_g, f1, of = stage_b(pb_blk, pb_h1u)
                    if debug_dump:
                        nc.sync.dma_start(out=dbg["d_h2u0"].ap(), in_=h2u_g[0][:, :])
                        nc.sync.dma_start(out=dbg["d_h2u1"].ap(), in_=h2u_g[1][:, :])
                        nc.sync.dma_start(out=dbg["d_f1"].ap(), in_=f1[:, :])
                        nc.sync.dma_start(out=dbg["d_of"].ap(), in_=of[:, :])
                pending = new_pending
    legalize_waits(nc)
    return nc


def kernel(**inputs):
    inputs = {k: np.asarray(v) for k, v in inputs.items()}
    d = {k: v.astype(np.float32) if v.dtype != np.float32 else v
         for k, v in inputs.items()}
    C = build_constants(d)
    packed = pack_constants(C)

    if 'nc' not in _cache:
        _cache['nc'] = build_bass()
    nc = _cache['nc']

    x = d['x'].reshape(B, 784)
    in_maps = []
    for c in range(NCORES):
        m = {'x': np.ascontiguousarray(x[c * BPC:(c + 1) * BPC])}
        for k, v in packed.items():
            m[k] = np.ascontiguousarray(v.astype(np.float32))
        in_maps.append(m)

    import os
    from concourse import bass_utils
    trace = os.environ.get("BASS_TRACE", "0") == "1"
    res = bass_utils.run_bass_kernel_spmd(nc, in_maps, core_ids=list(range(NCORES)),
                                          trace=trace)
    _cache['last_results'] = res
    out = np.concatenate([r['out'] for r in res.results], axis=0)
    return out.astype(np.float32)



# revision 37
# speedup vs baseline: 1.7311x; 1.7311x over previous
"""Binarized LeNet5 + BN (CIM-style) forward on 8 Trainium2 NeuronCores.

Pure data parallel: batch 8192 -> 1024 images/core, processed in 8 blocks of
128 images, fully on-chip per block, two-stage software pipeline across blocks.

Per-block stages (per core):
  A: DMA x-block -> XF [28,(img,28)] f32; PE broadcast SEL1 [28->120=(j,dy)];
     DVE is_ge -> XR {0,1} bf16; conv1 (2 o-chunks x 5 dx matmuls, K=120,
     M=72=(jp,j2,o)); fused BN+clip as t=min(max(z+c,lo),hi) on DVE; poolx
     DVE; pooly via signed pool-matrix matmul on PE; DVE is_ge -> H1u.
  B: PE broadcast SEL2_g [36x2->120=(j,c,dy)] -> XR2 {0,1}; conv2 (per g,
     5 dx matmuls K=120, M=64); same fused post; fc realign via selector
     matmuls; fc1/fc2 + ACT Sign; fc3 + ACT Identity; PE transpose; DMA out.

All weights/BN constants are folded host-side in numpy and passed as inputs.
The BN fold: y=clip(a*z+b,-1,1) with a=2*inv (step inputs) pools to
sign(sum y) = sign(sgn(a) * sum clip(z + b/a, -1/|a|, 1/|a|)); sgn(a) lives
in the pool matrix entries.
"""
import sys

sys.path.insert(0, "/opt/trn_rl_repo")

import numpy as np

NCORES = 8
B = 8192
BPC = B // NCORES          # images per core
NB = 128                   # images per block
NBLK = BPC // NB           # blocks per core
EPS = 1e-5

_cache = {}


def _binz(w):
    return np.where(w >= 0, 1.0, -1.0).astype(np.float32)


def _fp16r(x):
    # round-to-nearest fp16, back to f32 (pool-path clip bounds live in fp16)
    return np.asarray(x, np.float32).astype(np.float16).astype(np.float32)


def build_constants(d):
    C = {}
    # ---- conv1 ({0,1} input): h = 2*M - rowsum + b ----
    w1 = _binz(d['conv1_w'])
    lhsT1 = np.zeros((2, 5, 120, 72), np.float32)
    for chunk in range(2):
        for dx in range(5):
            for j in range(24):
                for dy in range(5):
                    p = j * 5 + dy
                    jp, j2 = j % 2, j // 2
                    m = jp * 36 + j2 * 3 + np.arange(3)
                    lhsT1[chunk, dx, p, m] = w1[chunk * 3 + np.arange(3), 0, dy, dx]
    C['lhsT1'] = lhsT1
    SEL1 = np.zeros((28, 120), np.float32)
    for j in range(24):
        for dy in range(5):
            SEL1[j + dy, j * 5 + dy] = 1.0
    C['SEL1'] = SEL1
    inv1 = d['bn1_g'] / np.sqrt(d['bn1_v'] + EPS)
    a1 = inv1          # conv1 input is +-1 (ACT Sign)
    b1 = inv1 * d['conv1_b'] + d['bn1_b'] - d['bn1_m'] * inv1
    c1 = b1 / a1
    lo1 = -1.0 / np.abs(a1)
    hi1 = 1.0 / np.abs(a1)
    sg1 = np.where(a1 >= 0, 1.0, -1.0).astype(np.float32)

    def expand72(vec6, chunk):
        out = np.zeros(72, np.float32)
        for jp in range(2):
            for j2 in range(12):
                out[jp * 36 + j2 * 3 + np.arange(3)] = vec6[chunk * 3 + np.arange(3)]
        return out

    L1 = _fp16r(lo1 - c1)
    H1 = _fp16r(hi1 - c1)
    # tie at full saturation must be exact: threshold = 2*(L+H)*sg in f32
    tau1 = (np.float32(2.0) * (L1 + H1) * sg1).astype(np.float32)
    C['c1'] = np.stack([expand72(c1, c) for c in range(2)])
    C['L1'] = np.stack([expand72(L1, c) for c in range(2)])
    C['H1'] = np.stack([expand72(H1, c) for c in range(2)])
    C['tau1'] = np.stack([expand72(tau1, c) for c in range(2)])
    C['sg1'] = np.stack([expand72(sg1, c) for c in range(2)])  # for poolm entries

    # ---- conv2 ({0,1} input) ----
    w2 = _binz(d['conv2_w'])
    SEL2 = np.zeros((2, 72, 120), np.float32)
    for g in range(2):
        for chunk in range(2):
            for j2 in range(12):
                for ol in range(3):
                    ps = chunk * 36 + j2 * 3 + ol
                    c = chunk * 3 + ol
                    for j in range(4):
                        for dy in range(5):
                            if j2 == 4 * g + j + dy:
                                SEL2[g, ps, j * 30 + c * 5 + dy] = 1.0
    C['SEL2'] = SEL2
    lhsT2 = np.zeros((5, 120, 64), np.float32)
    for dx in range(5):
        for j in range(4):
            for c in range(6):
                for dy in range(5):
                    p2 = j * 30 + c * 5 + dy
                    jp, j2 = j % 2, j // 2
                    lhsT2[dx, p2, jp * 32 + j2 * 16 + np.arange(16)] = w2[:, c, dy, dx]
    C['lhsT2'] = lhsT2
    inv2 = d['bn2_g'] / np.sqrt(d['bn2_v'] + EPS)
    rs2 = w2.reshape(16, -1).sum(1)
    a2 = 2.0 * inv2
    b2 = inv2 * (d['conv2_b'] - rs2) + d['bn2_b'] - d['bn2_m'] * inv2
    c2 = b2 / a2
    lo2 = -1.0 / np.abs(a2)
    hi2 = 1.0 / np.abs(a2)
    sg2 = np.where(a2 >= 0, 1.0, -1.0).astype(np.float32)

    def expand64(vec16):
        out = np.zeros(64, np.float32)
        for jp in range(2):
            for j2 in range(2):
                out[jp * 32 + j2 * 16 + np.arange(16)] = vec16
        return out

    L2 = _fp16r(lo2 - c2)
    H2 = _fp16r(hi2 - c2)
    tau2 = (np.float32(2.0) * (L2 + H2) * sg2).astype(np.float32)
    C['c2'] = expand64(c2)
    C['L2'] = expand64(L2)
    C['H2'] = expand64(H2)
    C['tau2'] = expand64(tau2)
    C['sg2'] = expand64(sg2)

    # ---- fc1 ({0,1} input) ----
    wf1 = _binz(d['fc1_w'])
    lf1 = np.zeros((2, 128, 120), np.float32)
    for t in range(2):
        for xl in range(2):
            xp = t * 2 + xl
            for g in range(2):
                for j2 in range(2):
                    y2 = 2 * g + j2
                    for o2 in range(16):
                        f = xl * 64 + g * 32 + j2 * 16 + o2
                        k = o2 * 16 + y2 * 4 + xp
                        lf1[t, f] = wf1[:, k]
    C['lhsTf1'] = lf1
    invf1 = d['bn_fc1_g'] / np.sqrt(d['bn_fc1_v'] + EPS)
    rsf1 = wf1.sum(1)
    C['bnf1_scale'] = (2.0 * invf1).astype(np.float32)
    C['bnf1_shift'] = (invf1 * (d['fc1_b'] - rsf1) + d['bn_fc1_b']
                      - d['bn_fc1_m'] * invf1).astype(np.float32)
    # ---- fc2 (+-1 input) ----
    wf2 = _binz(d['fc2_w'])
    C['lhsTf2'] = np.ascontiguousarray(wf2.T)
    invf2 = d['bn_fc2_g'] / np.sqrt(d['bn_fc2_v'] + EPS)
    C['bnf2_scale'] = invf2.astype(np.float32)
    C['bnf2_shift'] = (invf2 * d['fc2_b'] + d['bn_fc2_b']
                      - d['bn_fc2_m'] * invf2).astype(np.float32)
    # ---- fc3 (+-1 input) ----
    wf3 = _binz(d['fc3_w'])
    invf3 = d['bn_fc3_g'] / np.sqrt(d['bn_fc3_v'] + EPS)
    scale3 = invf3.astype(np.float32)
    shift3 = (invf3 * d['fc3_b'] + d['bn_fc3_b']
              - d['bn_fc3_m'] * invf3).astype(np.float32)
    # fused fc3+bn3+transpose: out[i, l] = sum_k f2[k, i] * (scale3[l]*w3[l,k])
    #                                      + 1 * shift3[l]  (rank-1 bias matmul)
    C['rhs3'] = (wf3.T * scale3[None, :]).astype(np.float32)   # [84, 10]
    C['rhs3s'] = shift3.reshape(1, 10).astype(np.float32)      # [1, 10]
    return C


def pack_constants(C):
    t = {}
    t['sel1'] = C['SEL1']                                          # [28,120] f32
    l1 = np.zeros((120, 720), np.float32)
    for chunk in range(2):
        for dx in range(5):
            l1[:, (chunk * 5 + dx) * 72:(chunk * 5 + dx + 1) * 72] = C['lhsT1'][chunk, dx]
    t['lhsT1'] = l1
    s2 = np.zeros((36, 480), np.float32)
    for g in range(2):
        for chunk in range(2):
            s2[:, (g * 2 + chunk) * 120:(g * 2 + chunk + 1) * 120] = \
                C['SEL2'][g][chunk * 36:(chunk + 1) * 36]
    t['sel2'] = s2
    # signed pool matrices: conv1 per chunk (sg differs by chunk)
    pm1 = np.zeros((72, 72), np.float32)   # cols: [chunk0 36 | chunk1 36]
    for chunk in range(2):
        for m in range(36):
            pm1[m, chunk * 36 + m] = C['sg1'][chunk][m]
            pm1[m + 36, chunk * 36 + m] = C['sg1'][chunk][m]
    t['poolm1'] = pm1
    pm2 = np.zeros((64, 32), np.float32)
    for m in range(32):
        pm2[m, m] = C['sg2'][m]
        pm2[m + 32, m] = C['sg2'][m]
    t['poolm2'] = pm2
    sf = np.zeros((32, 8 * 128), np.float32)
    for tt in range(2):
        for xl in range(2):
            for g in range(2):
                idx = (tt * 2 + xl) * 2 + g
                for s in range(32):
                    sf[s, idx * 128 + xl * 64 + g * 32 + s] = 1.0
    t['selfm'] = sf
    l2 = np.zeros((120, 320), np.float32)
    for dx in range(5):
        l2[:, dx * 64:(dx + 1) * 64] = C['lhsT2'][dx]
    t['lhsT2'] = l2
    lf = np.zeros((128, 240), np.float32)
    lf[:, :120] = C['lhsTf1'][0]
    lf[:, 120:] = C['lhsTf1'][1]
    t['lhsTf1'] = lf
    t['lhsTf2'] = C['lhsTf2']
    t['rhs3'] = C['rhs3']
    t['rhs3s'] = C['rhs3s']
    # bn vectors packed [128, 16] f32
    bn = np.zeros((128, 16), np.float32)
    bn[:72, 0] = C['L1'][0]; bn[:72, 1] = C['H1'][0]; bn[:72, 2] = C['tau1'][0]
    bn[:72, 3] = C['L1'][1]; bn[:72, 4] = C['H1'][1]; bn[:72, 5] = C['tau1'][1]
    bn[:64, 6] = C['L2'];    bn[:64, 7] = C['H2'];    bn[:64, 8] = C['tau2']
    bn[:120, 9] = C['bnf1_scale'];  bn[:120, 10] = C['bnf1_shift']
    bn[:84, 11] = C['bnf2_scale'];  bn[:84, 12] = C['bnf2_shift']
    t['bnv'] = bn
    return t


def legalize_waits(nc, max_waits=1):
    """Split multi-wait instructions: this walrus supports one sync wait per
    instruction, so move excess waits onto preceding same-engine NoOps."""
    import concourse.mybir as mybir
    n_split = 0
    for f in nc.m.functions:
        for b in f.blocks:
            new = []
            for inst in b.instructions:
                si = inst.sync_info
                if si is not None and len(si.on_wait) > max_waits:
                    waits = list(si.on_wait)
                    excess, keep = waits[:-max_waits], waits[-max_waits:]
                    for k, w in enumerate(excess):
                        nop = mybir.InstNoOp(name=f'{inst.name}-w{k}', ins=[], outs=[])
                        nop.engine = inst.engine
                        nop.sync_info = mybir.SyncInfo(on_wait=[w], on_update=[])
                        new.append(nop)
                    inst.sync_info = mybir.SyncInfo(on_wait=keep,
                                                    on_update=list(si.on_update))
                    n_split += 1
                new.append(inst)
            b.instructions[:] = new
    return n_split


def build_bass(debug_dump=False):
    import concourse.bass as bass
    import concourse.mybir as mybir
    from concourse.tile import TileContext

    f32 = mybir.dt.float32
    f32r = mybir.dt.float32r
    bf16 = mybir.dt.bfloat16
    fp16 = mybir.dt.float16
    AF = mybir.ActivationFunctionType
    OP = mybir.AluOpType

    nc = bass.Bass("TRN2", target_bir_lowering=False, debug=False,
                   enable_asserts=False, num_devices=NCORES)

    x_in = nc.dram_tensor("x", [BPC, 784], f32r, kind="ExternalInput")
    sel1_in = nc.dram_tensor("sel1", [28, 120], f32r, kind="ExternalInput")
    lhsT1_in = nc.dram_tensor("lhsT1", [120, 720], f32, kind="ExternalInput")
    sel2_in = nc.dram_tensor("sel2", [36, 480], f32, kind="ExternalInput")
    poolm1_in = nc.dram_tensor("poolm1", [72, 72], f32, kind="ExternalInput")
    poolm2_in = nc.dram_tensor("poolm2", [64, 32], f32, kind="ExternalInput")
    selfm_in = nc.dram_tensor("selfm", [32, 1024], f32, kind="ExternalInput")
    lhsT2_in = nc.dram_tensor("lhsT2", [120, 320], f32, kind="ExternalInput")
    lhsTf1_in = nc.dram_tensor("lhsTf1", [128, 240], f32, kind="ExternalInput")
    lhsTf2_in = nc.dram_tensor("lhsTf2", [120, 84], f32, kind="ExternalInput")
    rhs3_in = nc.dram_tensor("rhs3", [84, 10], f32, kind="ExternalInput")
    rhs3s_in = nc.dram_tensor("rhs3s", [1, 10], f32, kind="ExternalInput")
    bnv_in = nc.dram_tensor("bnv", [128, 16], f32, kind="ExternalInput")
    out_t = nc.dram_tensor("out", [BPC, 10], f32, kind="ExternalOutput")

    NI = 16               # conv1 images per matmul (16*24 = 384 cols)
    NI2 = 32              # conv2 images per matmul (32*8 = 256 cols)

    with TileContext(nc) as tc:
        with (
            tc.tile_pool(name="consts", bufs=1) as cpool,
            tc.tile_pool(name="xf", bufs=2) as xfpool,
            tc.tile_pool(name="xr", bufs=2) as xrpool,
            tc.tile_pool(name="work", bufs=3) as wpool,
            tc.tile_pool(name="h1u", bufs=2) as h1pool,
            tc.tile_pool(name="xr2", bufs=2) as xr2pool,
            tc.tile_pool(name="fc", bufs=2) as fcpool,
            tc.tile_pool(name="psA", bufs=2, space="PSUM") as psA,
            tc.tile_pool(name="psB", bufs=2, space="PSUM") as psB,
            tc.tile_pool(name="psC", bufs=1, space="PSUM") as psC,
        ):
            def load_const(name, src_t, shape, dtype):
                if dtype in (bf16, fp16):
                    tf = cpool.tile(shape, f32, tag=name + "f", name=name + "f")
                    nc.gpsimd.dma_start(out=tf[:, :], in_=src_t.ap())
                    tl = cpool.tile(shape, dtype, tag=name, name=name)
                    nc.vector.tensor_copy(tl[:, :], tf[:, :])
                    return tl
                tl = cpool.tile(shape, dtype, tag=name, name=name)
                nc.gpsimd.dma_start(out=tl[:, :], in_=src_t.ap())
                return tl

            sel1 = load_const("sel1", sel1_in, [28, 120], f32r)
            lhsT1 = load_const("lhsT1", lhsT1_in, [120, 720], bf16)
            sel2 = load_const("sel2", sel2_in, [36, 480], bf16)
            poolm1 = load_const("poolm1", poolm1_in, [72, 72], fp16)
            poolm2 = load_const("poolm2", poolm2_in, [64, 32], fp16)
            selfm = load_const("selfm", selfm_in, [32, 1024], bf16)
            lhsT2 = load_const("lhsT2", lhsT2_in, [120, 320], bf16)
            lhsTf1 = load_const("lhsTf1", lhsTf1_in, [128, 240], bf16)
            lhsTf2 = load_const("lhsTf2", lhsTf2_in, [120, 84], bf16)
            rhs3 = load_const("rhs3", rhs3_in, [84, 10], fp16)
            rhs3s = load_const("rhs3s", rhs3s_in, [1, 10], fp16)
            ones1 = cpool.tile([1, NB], fp16, tag="ones1", name="ones1")
            nc.vector.memset(ones1[:, :], 1.0)
            bnv = load_const("bnv", bnv_in, [128, 16], f32)

            def stage_a(blk):
                """DMA + bcast1 + binarize + conv1 + pool -> h1u tiles."""
                xf = xfpool.tile([28, NB * 28], f32r, tag="xf", name="xf")
                src = x_in.ap()[blk * NB:(blk + 1) * NB, :].rearrange(
                    "i (r x) -> r i x", x=28)
                nc.gpsimd.dma_start(
                    out=xf[:, :].rearrange("r (i x) -> r i x", x=28), in_=src)

                xr = xrpool.tile([120, NB * 28], bf16, tag="xr", name="xr")
                for c0 in range(0, NB * 28, 448):
                    pb = psA.tile([120, 448], f32, tag="bcast", name="pb")
                    nc.tensor.matmul(pb[:, :], sel1[:, :], xf[:, c0:c0 + 448],
                                     start=True, stop=True)
                    nc.scalar.activation(xr[:, c0:c0 + 448], pb[:, :], AF.Sign)

                h1u_c = [h1pool.tile([36, NB * 12], bf16, tag=f"h1u{c}",
                                     name=f"h1u{c}") for c in range(2)]
                xrv = xr[:, :].rearrange("p (i x) -> p i x", x=28)
                for chunk in range(2):
                    svs = []
                    for pair in range(4):          # pairs of NI-image groups
                        ps = psB.tile([72, 1024], f32, tag="cps", name="c1ps")
                        for sub in range(2):
                            i0 = (pair * 2 + sub) * NI
                            for dx in range(5):
                                rhs = xrv[:, i0:i0 + NI, dx:dx + 24]
                                nc.tensor.matmul(
                                    ps[:, sub * 512:sub * 512 + NI * 24],
                                    lhsT1[:, (chunk * 5 + dx) * 72:(chunk * 5 + dx + 1) * 72],
                                    rhs, start=(dx == 0), stop=(dx == 4))
                        # q = clip2(z, L, H): one fused (max, min) op over both halves
                        s = wpool.tile([72, 2 * NI * 24], fp16, tag="s1", name="s1")
                        psv = ps[:, :].rearrange("p (two n) -> p two n", two=2)
                        sv = s[:, :].rearrange("p (two n) -> p two n", two=2)
                        nc.vector.tensor_scalar(
                            sv[:, :, :NI * 24], psv[:, :, :NI * 24],
                            bnv[:72, 3 * chunk:3 * chunk + 1],
                            bnv[:72, 3 * chunk + 1:3 * chunk + 2],
                            op0=OP.max, op1=OP.min)
                        svs.append(s)
                    # pool x+y in one PE pass pair: 2 accumulating matmuls with
                    # stride-2 column APs; then threshold on DVE
                    for pair in range(4):
                        s = svs[pair]
                        sq = s[:, :].rearrange("p (sub i x2 xp) -> p sub i x2 xp",
                                               sub=2, i=NI, xp=2)
                        psy = psB.tile([36, 2 * NI * 12], f32, tag="cpy", name="pyps", bufs=1)
                        pyv = psy[:, :].rearrange("p (sub i x2) -> p sub i x2",
                                                  sub=2, i=NI)
                        for xp in range(2):
                            nc.tensor.matmul(pyv,
                                             poolm1[:, 36 * chunk:36 * (chunk + 1)],
                                             sq[:, :, :, :, xp],
                                             start=(xp == 0), stop=(xp == 1))
                        nc.vector.tensor_scalar(
                            h1u_c[chunk][:, pair * 384:(pair + 1) * 384],
                            psy[:, :], bnv[:36, 3 * chunk + 2:3 * chunk + 3],
                            None, op0=OP.is_ge)
                return xr, h1u_c


            def stage_b(blk, h1u_c):
                """bcast2 + conv2 + pool + fc + out."""
                xr2 = xr2pool.tile([120, 2 * NB * 12], bf16, tag="xr2", name="xr2")
                for g in range(2):
                    ncols2 = NB * 12
                    for c0 in range(0, ncols2, 384):
                        pb = psA.tile([120, 448], f32, tag="bcast", name="pb2")
                        for chunk in range(2):
                            nc.tensor.matmul(
                                pb[:, :384],
                                sel2[:, (g * 2 + chunk) * 120:(g * 2 + chunk + 1) * 120],
                                h1u_c[chunk][:, c0:c0 + 384],
                                start=(chunk == 0), stop=(chunk == 1))
                        nc.scalar.activation(
                            xr2[:, g * ncols2 + c0:g * ncols2 + c0 + 384],
                            pb[:, :384], AF.Copy)

                h2u_g = [fcpool.tile([32, NB * 4], bf16, tag=f"h2u{g}",
                                     name=f"h2u{g}") for g in range(2)]
                for g in range(2):
                    xr2v = xr2[:, g * NB * 12:(g + 1) * NB * 12].rearrange(
                        "p (i x) -> p i x", x=12)
                    svs = []
                    for pair in range(2):
                        ps = psB.tile([64, 1024], f32, tag="cps", name="c2ps")
                        for sub in range(2):
                            i0 = (pair * 2 + sub) * NI2
                            for dx in range(5):
                                rhs = xr2v[:, i0:i0 + NI2, dx:dx + 8]
                                nc.tensor.matmul(
                                    ps[:, sub * 512:sub * 512 + NI2 * 8],
                                    lhsT2[:, dx * 64:(dx + 1) * 64],
                                    rhs, start=(dx == 0), stop=(dx == 4))
                        s = wpool.tile([64, 2 * NI2 * 8], fp16, tag="s2", name="s2")
                        psv = ps[:, :].rearrange("p (two n) -> p two n", two=2)
                        sv = s[:, :].rearrange("p (two n) -> p two n", two=2)
                        nc.vector.tensor_scalar(
                            sv[:, :, :NI2 * 8], psv[:, :, :NI2 * 8],
                            bnv[:64, 6:7], bnv[:64, 7:8], op0=OP.max, op1=OP.min)
                        svs.append(s)
                    for pair in range(2):
                        s = svs[pair]
                        sq = s[:, :].rearrange("p (sub i x2 xp) -> p sub i x2 xp",
                                               sub=2, i=NI2, xp=2)
                        psy = psB.tile([32, 2 * NI2 * 4], f32, tag="cpy", name="py2ps", bufs=1)
                        pyv = psy[:, :].rearrange("p (sub i x2) -> p sub i x2",
                                                  sub=2, i=NI2)
                        for xp in range(2):
                            nc.tensor.matmul(pyv, poolm2[:, :], sq[:, :, :, :, xp],
                                             start=(xp == 0), stop=(xp == 1))
                        nc.vector.tensor_scalar(
                            h2u_g[g][:, pair * 256:(pair + 1) * 256],
                            psy[:, :], bnv[:32, 8:9], None, op0=OP.is_ge)

                # FC stage
                rf = [fcpool.tile([128, NB], bf16, tag=f"rf{t}", name=f"rf{t}")
                      for t in range(2)]
                h2v = [h2u_g[g][:, :].rearrange("p (i x) -> p i x", x=4)
                       for g in range(2)]
                for t in range(2):
                    prf = psC.tile([128, NB], f32, tag="fcps", name="prf")
                    k = 0
                    for xl in range(2):
                        for g in range(2):
                            idx = (t * 2 + xl) * 2 + g
                            nc.tensor.matmul(
                                prf[:, :], selfm[:, idx * 128:(idx + 1) * 128],
                                h2v[g][:, :, t * 2 + xl],
                                start=(k == 0), stop=(k == 3))
                            k += 1
                    nc.scalar.activation(rf[t][:, :], prf[:, :], AF.Copy)
                psf = psC.tile([120, NB], f32, tag="fcps", name="psf")
                nc.tensor.matmul(psf[:, :], lhsTf1[:, 0:120], rf[0][:, :],
                                 start=True, stop=False)
                nc.tensor.matmul(psf[:, :], lhsTf1[:, 120:240], rf[1][:, :],
                                 start=False, stop=True)
                f1 = fcpool.tile([120, NB], bf16, tag="f1", name="f1")
                nc.scalar.activation(f1[:, :], psf[:, :], AF.Sign,
                                     bias=bnv[:120, 10:11], scale=bnv[:120, 9:10])
                psf2 = psC.tile([84, NB], f32, tag="fcps", name="psf2")
                nc.tensor.matmul(psf2[:, :], lhsTf2[:, :], f1[:, :],
                                 start=True, stop=True)
                f2 = fcpool.tile([84, NB], fp16, tag="f2", name="f2")
                nc.scalar.activation(f2[:, :], psf2[:, :], AF.Sign,
                                     bias=bnv[:84, 12:13], scale=bnv[:84, 11:12])
                # fused fc3+bn3+transpose: stationary = f2 data (84xNB),
                # moving = rhs3 (84x10) -> psum [NB, 10] already transposed;
                # bias via rank-1 accumulate (ones x shift).
                pso = psC.tile([NB, 10], f32, tag="fcps", name="pso")
                nc.tensor.matmul(pso[:, :], f2[:, :], rhs3[:, :],
                                 start=True, stop=False)
                nc.tensor.matmul(pso[:, :], ones1[:, :], rhs3s[:, :],
                                 start=False, stop=True)
                ot = fcpool.tile([NB, 10], f32, tag="ot", name="ot")
                nc.scalar.activation(ot[:, :], pso[:, :], AF.Copy)
                nc.sync.dma_start(out=out_t.ap()[blk * NB:(blk + 1) * NB, :],
                                  in_=ot[:, :])
                return h2u_g, f1, ot

            # two-stage software pipeline over blocks
            pending = None
            for blk in range(NBLK + 1):
                new_pending = None
                if blk < NBLK:
                    xr, h1u_c = stage_a(blk)
                    new_pending = (blk, h1u_c)
                if pending is not None:
                    stage_b(*pending)
                pending = new_pending
    legalize_waits(nc)
    return nc


def kernel(**inputs):
    inputs = {k: np.asarray(v) for k, v in inputs.items()}
    d = {k: v.astype(np.float32) if v.dtype != np.float32 else v
         for k, v in inputs.items()}
    C = build_constants(d)
    packed = pack_constants(C)

    if 'nc' not in _cache:
        _cache['nc'] = build_bass()
    nc = _cache['nc']

    x = d['x'].reshape(B, 784)
    in_maps = []
    for c in range(NCORES):
        m = {'x': np.ascontiguousarray(x[c * BPC:(c + 1) * BPC])}
        for k, v in packed.items():
            m[k] = np.ascontiguousarray(v.astype(np.float32))
        in_maps.append(m)

    import os
    from concourse import bass_utils
    trace = os.environ.get("BASS_TRACE", "0") == "1"
    res = bass_utils.run_bass_kernel_spmd(nc, in_maps, core_ids=list(range(NCORES)),
                                          trace=trace)
    _cache['last_results'] = res
    out = np.concatenate([r['out'] for r in res.results], axis=0)
    return out.astype(np.float32)



# revision 57
# speedup vs baseline: 1.7554x; 1.0140x over previous
"""Binarized LeNet5 + BN (CIM-style) forward on 8 Trainium2 NeuronCores.

Pure data parallel: batch 8192 -> 1024 images/core, processed in 8 blocks of
128 images, fully on-chip per block, two-stage software pipeline across blocks.

Per-block stages (per core):
  A: DMA x-block -> XF [28,(img,28)] f32; PE broadcast SEL1 [28->120=(j,dy)];
     DVE is_ge -> XR {0,1} bf16; conv1 (2 o-chunks x 5 dx matmuls, K=120,
     M=72=(jp,j2,o)); fused BN+clip as t=min(max(z+c,lo),hi) on DVE; poolx
     DVE; pooly via signed pool-matrix matmul on PE; DVE is_ge -> H1u.
  B: PE broadcast SEL2_g [36x2->120=(j,c,dy)] -> XR2 {0,1}; conv2 (per g,
     5 dx matmuls K=120, M=64); same fused post; fc realign via selector
     matmuls; fc1/fc2 + ACT Sign; fc3 + ACT Identity; PE transpose; DMA out.

All weights/BN constants are folded host-side in numpy and passed as inputs.
The BN fold: y=clip(a*z+b,-1,1) with a=2*inv (step inputs) pools to
sign(sum y) = sign(sgn(a) * sum clip(z + b/a, -1/|a|, 1/|a|)); sgn(a) lives
in the pool matrix entries.
"""
import sys

sys.path.insert(0, "/opt/trn_rl_repo")

import numpy as np

NCORES = 8
B = 8192
BPC = B // NCORES          # images per core
NB = 128                   # images per block
NBLK = BPC // NB           # blocks per core
EPS = 1e-5

_cache = {}


def _binz(w):
    return np.where(w >= 0, 1.0, -1.0).astype(np.float32)


def _fp16r(x):
    # round-to-nearest fp16, back to f32 (pool-path clip bounds live in fp16)
    return np.asarray(x, np.float32).astype(np.float16).astype(np.float32)


def build_constants(d):
    C = {}
    # ---- conv1 ({0,1} input): h = 2*M - rowsum + b ----
    w1 = _binz(d['conv1_w'])
    lhsT1 = np.zeros((2, 5, 120, 72), np.float32)
    for chunk in range(2):
        for dx in range(5):
            for j in range(24):
                for dy in range(5):
                    p = j * 5 + dy
                    jp, j2 = j % 2, j // 2
                    m = jp * 36 + j2 * 3 + np.arange(3)
                    lhsT1[chunk, dx, p, m] = w1[chunk * 3 + np.arange(3), 0, dy, dx]
    C['lhsT1'] = lhsT1
    SEL1 = np.zeros((28, 120), np.float32)
    for j in range(24):
        for dy in range(5):
            SEL1[j + dy, j * 5 + dy] = 1.0
    C['SEL1'] = SEL1
    inv1 = d['bn1_g'] / np.sqrt(d['bn1_v'] + EPS)
    a1 = inv1          # conv1 input is +-1 (ACT Sign)
    b1 = inv1 * d['conv1_b'] + d['bn1_b'] - d['bn1_m'] * inv1
    c1 = b1 / a1
    lo1 = -1.0 / np.abs(a1)
    hi1 = 1.0 / np.abs(a1)
    sg1 = np.where(a1 >= 0, 1.0, -1.0).astype(np.float32)

    def expand72(vec6, chunk):
        out = np.zeros(72, np.float32)
        for jp in range(2):
            for j2 in range(12):
                out[jp * 36 + j2 * 3 + np.arange(3)] = vec6[chunk * 3 + np.arange(3)]
        return out

    L1 = _fp16r(lo1 - c1)
    H1 = _fp16r(hi1 - c1)
    # tie at full saturation must be exact: threshold = 2*(L+H)*sg in f32
    tau1 = (np.float32(2.0) * (L1 + H1) * sg1).astype(np.float32)
    C['c1'] = np.stack([expand72(c1, c) for c in range(2)])
    C['L1'] = np.stack([expand72(L1, c) for c in range(2)])
    C['H1'] = np.stack([expand72(H1, c) for c in range(2)])
    C['tau1'] = np.stack([expand72(tau1, c) for c in range(2)])
    C['sg1'] = np.stack([expand72(sg1, c) for c in range(2)])  # for poolm entries

    # ---- conv2 ({0,1} input) ----
    w2 = _binz(d['conv2_w'])
    SEL2 = np.zeros((2, 72, 120), np.float32)
    for g in range(2):
        for chunk in range(2):
            for j2 in range(12):
                for ol in range(3):
                    ps = chunk * 36 + j2 * 3 + ol
                    c = chunk * 3 + ol
                    for j in range(4):
                        for dy in range(5):
                            if j2 == 4 * g + j + dy:
                                SEL2[g, ps, j * 30 + c * 5 + dy] = 1.0
    C['SEL2'] = SEL2
    lhsT2 = np.zeros((5, 120, 64), np.float32)
    for dx in range(5):
        for j in range(4):
            for c in range(6):
                for dy in range(5):
                    p2 = j * 30 + c * 5 + dy
                    jp, j2 = j % 2, j // 2
                    lhsT2[dx, p2, jp * 32 + j2 * 16 + np.arange(16)] = w2[:, c, dy, dx]
    C['lhsT2'] = lhsT2
    inv2 = d['bn2_g'] / np.sqrt(d['bn2_v'] + EPS)
    rs2 = w2.reshape(16, -1).sum(1)
    a2 = 2.0 * inv2
    b2 = inv2 * (d['conv2_b'] - rs2) + d['bn2_b'] - d['bn2_m'] * inv2
    c2 = b2 / a2
    lo2 = -1.0 / np.abs(a2)
    hi2 = 1.0 / np.abs(a2)
    sg2 = np.where(a2 >= 0, 1.0, -1.0).astype(np.float32)

    def expand64(vec16):
        out = np.zeros(64, np.float32)
        for jp in range(2):
            for j2 in range(2):
                out[jp * 32 + j2 * 16 + np.arange(16)] = vec16
        return out

    L2 = _fp16r(lo2 - c2)
    H2 = _fp16r(hi2 - c2)
    tau2 = (np.float32(2.0) * (L2 + H2) * sg2).astype(np.float32)
    C['c2'] = expand64(c2)
    C['L2'] = expand64(L2)
    C['H2'] = expand64(H2)
    C['tau2'] = expand64(tau2)
    C['sg2'] = expand64(sg2)

    # ---- fc1 ({0,1} input) ----
    wf1 = _binz(d['fc1_w'])
    lf1 = np.zeros((2, 128, 120), np.float32)
    for t in range(2):
        for xl in range(2):
            xp = t * 2 + xl
            for g in range(2):
                for j2 in range(2):
                    y2 = 2 * g + j2
                    for o2 in range(16):
                        f = xl * 64 + g * 32 + j2 * 16 + o2
                        k = o2 * 16 + y2 * 4 + xp
                        lf1[t, f] = wf1[:, k]
    C['lhsTf1'] = lf1
    invf1 = d['bn_fc1_g'] / np.sqrt(d['bn_fc1_v'] + EPS)
    rsf1 = wf1.sum(1)
    C['bnf1_scale'] = (2.0 * invf1).astype(np.float32)
    C['bnf1_shift'] = (invf1 * (d['fc1_b'] - rsf1) + d['bn_fc1_b']
                      - d['bn_fc1_m'] * invf1).astype(np.float32)
    # ---- fc2 (+-1 input) ----
    wf2 = _binz(d['fc2_w'])
    C['lhsTf2'] = np.ascontiguousarray(wf2.T)
    invf2 = d['bn_fc2_g'] / np.sqrt(d['bn_fc2_v'] + EPS)
    C['bnf2_scale'] = invf2.astype(np.float32)
    C['bnf2_shift'] = (invf2 * d['fc2_b'] + d['bn_fc2_b']
                      - d['bn_fc2_m'] * invf2).astype(np.float32)
    # ---- fc3 (+-1 input) ----
    wf3 = _binz(d['fc3_w'])
    invf3 = d['bn_fc3_g'] / np.sqrt(d['bn_fc3_v'] + EPS)
    scale3 = invf3.astype(np.float32)
    shift3 = (invf3 * d['fc3_b'] + d['bn_fc3_b']
              - d['bn_fc3_m'] * invf3).astype(np.float32)
    # fused fc3+bn3+transpose: out[i, l] = sum_k f2[k, i] * (scale3[l]*w3[l,k])
    #                                      + 1 * shift3[l]  (rank-1 bias matmul)
    C['rhs3'] = (wf3.T * scale3[None, :]).astype(np.float32)   # [84, 10]
    C['rhs3s'] = shift3.reshape(1, 10).astype(np.float32)      # [1, 10]
    return C


def pack_constants(C):
    t = {}
    t['sel1'] = C['SEL1']                                          # [28,120] f32
    l1 = np.zeros((120, 720), np.float32)
    for chunk in range(2):
        for dx in range(5):
            l1[:, (chunk * 5 + dx) * 72:(chunk * 5 + dx + 1) * 72] = C['lhsT1'][chunk, dx]
    t['lhsT1'] = l1
    s2 = np.zeros((36, 480), np.float32)
    for g in range(2):
        for chunk in range(2):
            s2[:, (g * 2 + chunk) * 120:(g * 2 + chunk + 1) * 120] = \
                C['SEL2'][g][chunk * 36:(chunk + 1) * 36]
    t['sel2'] = s2
    # signed pool matrices: conv1 per chunk (sg differs by chunk)
    pm1 = np.zeros((72, 72), np.float32)   # cols: [chunk0 36 | chunk1 36]
    for chunk in range(2):
        for m in range(36):
            pm1[m, chunk * 36 + m] = C['sg1'][chunk][m]
            pm1[m + 36, chunk * 36 + m] = C['sg1'][chunk][m]
    t['poolm1'] = pm1
    pm2 = np.zeros((64, 32), np.float32)
    for m in range(32):
        pm2[m, m] = C['sg2'][m]
        pm2[m + 32, m] = C['sg2'][m]
    t['poolm2'] = pm2
    sf = np.zeros((32, 8 * 128), np.float32)
    for tt in range(2):
        for xl in range(2):
            for g in range(2):
                idx = (tt * 2 + xl) * 2 + g
                for s in range(32):
                    sf[s, idx * 128 + xl * 64 + g * 32 + s] = 1.0
    t['selfm'] = sf
    l2 = np.zeros((120, 320), np.float32)
    for dx in range(5):
        l2[:, dx * 64:(dx + 1) * 64] = C['lhsT2'][dx]
    t['lhsT2'] = l2
    lf = np.zeros((128, 240), np.float32)
    lf[:, :120] = C['lhsTf1'][0]
    lf[:, 120:] = C['lhsTf1'][1]
    t['lhsTf1'] = lf
    t['lhsTf2'] = C['lhsTf2']
    t['rhs3'] = C['rhs3']
    t['rhs3s'] = C['rhs3s']
    # bn vectors packed [128, 16] f32
    bn = np.zeros((128, 16), np.float32)
    bn[:72, 0] = C['L1'][0]; bn[:72, 1] = C['H1'][0]; bn[:72, 2] = C['tau1'][0]
    bn[:72, 3] = C['L1'][1]; bn[:72, 4] = C['H1'][1]; bn[:72, 5] = C['tau1'][1]
    bn[:64, 6] = C['L2'];    bn[:64, 7] = C['H2'];    bn[:64, 8] = C['tau2']
    bn[:120, 9] = C['bnf1_scale'];  bn[:120, 10] = C['bnf1_shift']
    bn[:84, 11] = C['bnf2_scale'];  bn[:84, 12] = C['bnf2_shift']
    t['bnv'] = bn
    return t


def legalize_waits(nc, max_waits=1):
    """Split multi-wait instructions: this walrus supports one sync wait per
    instruction, so move excess waits onto preceding same-engine NoOps."""
    import concourse.mybir as mybir
    n_split = 0
    for f in nc.m.functions:
        for b in f.blocks:
            new = []
            for inst in b.instructions:
                si = inst.sync_info
                if si is not None and len(si.on_wait) > max_waits:
                    waits = list(si.on_wait)
                    excess, keep = waits[:-max_waits], waits[-max_waits:]
                    for k, w in enumerate(excess):
                        nop = mybir.InstNoOp(name=f'{inst.name}-w{k}', ins=[], outs=[])
                        nop.engine = inst.engine
                        nop.sync_info = mybir.SyncInfo(on_wait=[w], on_update=[])
                        new.append(nop)
                    inst.sync_info = mybir.SyncInfo(on_wait=keep,
                                                    on_update=list(si.on_update))
                    n_split += 1
                new.append(inst)
            b.instructions[:] = new
    return n_split


def build_bass(debug_dump=False):
    import concourse.bass as bass
    import concourse.mybir as mybir
    from concourse.tile import TileContext

    f32 = mybir.dt.float32
    f32r = mybir.dt.float32r
    bf16 = mybir.dt.bfloat16
    fp16 = mybir.dt.float16
    AF = mybir.ActivationFunctionType
    OP = mybir.AluOpType

    nc = bass.Bass("TRN2", target_bir_lowering=False, debug=False,
                   enable_asserts=False, num_devices=NCORES)

    x_in = nc.dram_tensor("x", [BPC, 784], f32r, kind="ExternalInput")
    sel1_in = nc.dram_tensor("sel1", [28, 120], f32r, kind="ExternalInput")
    lhsT1_in = nc.dram_tensor("lhsT1", [120, 720], f32, kind="ExternalInput")
    sel2_in = nc.dram_tensor("sel2", [36, 480], f32, kind="ExternalInput")
    poolm1_in = nc.dram_tensor("poolm1", [72, 72], f32, kind="ExternalInput")
    poolm2_in = nc.dram_tensor("poolm2", [64, 32], f32, kind="ExternalInput")
    selfm_in = nc.dram_tensor("selfm", [32, 1024], f32, kind="ExternalInput")
    lhsT2_in = nc.dram_tensor("lhsT2", [120, 320], f32, kind="ExternalInput")
    lhsTf1_in = nc.dram_tensor("lhsTf1", [128, 240], f32, kind="ExternalInput")
    lhsTf2_in = nc.dram_tensor("lhsTf2", [120, 84], f32, kind="ExternalInput")
    rhs3_in = nc.dram_tensor("rhs3", [84, 10], f32, kind="ExternalInput")
    rhs3s_in = nc.dram_tensor("rhs3s", [1, 10], f32, kind="ExternalInput")
    bnv_in = nc.dram_tensor("bnv", [128, 16], f32, kind="ExternalInput")
    out_t = nc.dram_tensor("out", [BPC, 10], f32, kind="ExternalOutput")

    NI = 16               # conv1 images per matmul (16*24 = 384 cols)
    NI2 = 32              # conv2 images per matmul (32*8 = 256 cols)

    with TileContext(nc) as tc:
        with (
            tc.tile_pool(name="consts", bufs=1) as cpool,
            tc.tile_pool(name="xf", bufs=2) as xfpool,
            tc.tile_pool(name="xr", bufs=2) as xrpool,
            tc.tile_pool(name="work", bufs=3) as wpool,
            tc.tile_pool(name="h1u", bufs=2) as h1pool,
            tc.tile_pool(name="xr2", bufs=2) as xr2pool,
            tc.tile_pool(name="fc", bufs=2) as fcpool,
            tc.tile_pool(name="psA", bufs=2, space="PSUM") as psA,
            tc.tile_pool(name="psB", bufs=2, space="PSUM") as psB,
            tc.tile_pool(name="psC", bufs=1, space="PSUM") as psC,
        ):
            def load_const(name, src_t, shape, dtype):
                if dtype in (bf16, fp16):
                    tf = cpool.tile(shape, f32, tag=name + "f", name=name + "f")
                    nc.gpsimd.dma_start(out=tf[:, :], in_=src_t.ap())
                    tl = cpool.tile(shape, dtype, tag=name, name=name)
                    nc.vector.tensor_copy(tl[:, :], tf[:, :])
                    return tl
                tl = cpool.tile(shape, dtype, tag=name, name=name)
                nc.gpsimd.dma_start(out=tl[:, :], in_=src_t.ap())
                return tl

            sel1 = load_const("sel1", sel1_in, [28, 120], f32r)
            lhsT1 = load_const("lhsT1", lhsT1_in, [120, 720], bf16)
            sel2 = load_const("sel2", sel2_in, [36, 480], bf16)
            poolm1 = load_const("poolm1", poolm1_in, [72, 72], fp16)
            poolm2 = load_const("poolm2", poolm2_in, [64, 32], fp16)
            selfm = load_const("selfm", selfm_in, [32, 1024], bf16)
            lhsT2 = load_const("lhsT2", lhsT2_in, [120, 320], bf16)
            lhsTf1 = load_const("lhsTf1", lhsTf1_in, [128, 240], bf16)
            lhsTf2 = load_const("lhsTf2", lhsTf2_in, [120, 84], bf16)
            rhs3 = load_const("rhs3", rhs3_in, [84, 10], fp16)
            rhs3s = load_const("rhs3s", rhs3s_in, [1, 10], fp16)
            ones1 = cpool.tile([1, NB], fp16, tag="ones1", name="ones1")
            nc.vector.memset(ones1[:, :], 1.0)
            bnv = load_const("bnv", bnv_in, [128, 16], f32)

            def stage_a(blk):
                """DMA + bcast1 + binarize + conv1 + pool -> h1u tiles."""
                xf = xfpool.tile([28, NB * 28], f32r, tag="xf", name="xf")
                src = x_in.ap()[blk * NB:(blk + 1) * NB, :].rearrange(
                    "i (r x) -> r i x", x=28)
                nc.gpsimd.dma_start(
                    out=xf[:, :].rearrange("r (i x) -> r i x", x=28), in_=src)

                xr = xrpool.tile([120, NB * 28], bf16, tag="xr", name="xr")
                for c0 in range(0, NB * 28, 448):
                    pb = psA.tile([120, 448], f32, tag="bcast", name="pb")
                    nc.tensor.matmul(pb[:, :], sel1[:, :], xf[:, c0:c0 + 448],
                                     start=True, stop=True)
                    nc.scalar.activation(xr[:, c0:c0 + 448], pb[:, :], AF.Sign)

                h1u_c = [h1pool.tile([36, NB * 12], bf16, tag=f"h1u{c}",
                                     name=f"h1u{c}") for c in range(2)]
                xrv = xr[:, :].rearrange("p (i x) -> p i x", x=28)
                for chunk in range(2):
                    svs = []
                    for pair in range(4):          # pairs of NI-image groups
                        ps = psB.tile([72, 1024], f32, tag="cps", name="c1ps")
                        for sub in range(2):
                            i0 = (pair * 2 + sub) * NI
                            for dx in range(5):
                                rhs = xrv[:, i0:i0 + NI, dx:dx + 24]
                                nc.tensor.matmul(
                                    ps[:, sub * 512:sub * 512 + NI * 24],
                                    lhsT1[:, (chunk * 5 + dx) * 72:(chunk * 5 + dx + 1) * 72],
                                    rhs, start=(dx == 0), stop=(dx == 4))
                        # q = clip2(z, L, H): one fused (max, min) op over both halves
                        s = wpool.tile([72, 2 * NI * 24], fp16, tag="s1", name="s1")
                        psv = ps[:, :].rearrange("p (two n) -> p two n", two=2)
                        sv = s[:, :].rearrange("p (two n) -> p two n", two=2)
                        nc.vector.tensor_scalar(
                            sv[:, :, :NI * 24], psv[:, :, :NI * 24],
                            bnv[:72, 3 * chunk:3 * chunk + 1],
                            bnv[:72, 3 * chunk + 1:3 * chunk + 2],
                            op0=OP.max, op1=OP.min)
                        svs.append(s)
                    # pool x+y in one PE pass pair: 2 accumulating matmuls with
                    # stride-2 column APs; then threshold on DVE
                    for pair in range(4):
                        s = svs[pair]
                        sq = s[:, :].rearrange("p (sub i x2 xp) -> p sub i x2 xp",
                                               sub=2, i=NI, xp=2)
                        psy = psB.tile([36, 2 * NI * 12], f32, tag="cpy", name="pyps", bufs=1)
                        pyv = psy[:, :].rearrange("p (sub i x2) -> p sub i x2",
                                                  sub=2, i=NI)
                        for xp in range(2):
                            nc.tensor.matmul(pyv,
                                             poolm1[:, 36 * chunk:36 * (chunk + 1)],
                                             sq[:, :, :, :, xp],
                                             start=(xp == 0), stop=(xp == 1))
                        nc.vector.tensor_scalar(
                            h1u_c[chunk][:, pair * 384:(pair + 1) * 384],
                            psy[:, :], bnv[:36, 3 * chunk + 2:3 * chunk + 3],
                            None, op0=OP.is_ge)
                return xr, h1u_c


            def stage_b(blk, h1u_c):
                """bcast2 + conv2 + pool + fc + out."""
                xr2 = xr2pool.tile([120, 2 * NB * 12], bf16, tag="xr2", name="xr2")
                for g in range(2):
                    ncols2 = NB * 12
                    for c0 in range(0, ncols2, 384):
                        pb = psA.tile([120, 448], f32, tag="bcast", name="pb2")
                        for chunk in range(2):
                            nc.tensor.matmul(
                                pb[:, :384],
                                sel2[:, (g * 2 + chunk) * 120:(g * 2 + chunk + 1) * 120],
                                h1u_c[chunk][:, c0:c0 + 384],
                                start=(chunk == 0), stop=(chunk == 1))
                        nc.scalar.activation(
                            xr2[:, g * ncols2 + c0:g * ncols2 + c0 + 384],
                            pb[:, :384], AF.Copy)

                h2u_g = [fcpool.tile([32, NB * 4], bf16, tag=f"h2u{g}",
                                     name=f"h2u{g}") for g in range(2)]
                for g in range(2):
                    xr2v = xr2[:, g * NB * 12:(g + 1) * NB * 12].rearrange(
                        "p (i x) -> p i x", x=12)
                    svs = []
                    for pair in range(2):
                        ps = psB.tile([64, 1024], f32, tag="cps", name="c2ps")
                        for sub in range(2):
                            i0 = (pair * 2 + sub) * NI2
                            for dx in range(5):
                                rhs = xr2v[:, i0:i0 + NI2, dx:dx + 8]
                                nc.tensor.matmul(
                                    ps[:, sub * 512:sub * 512 + NI2 * 8],
                                    lhsT2[:, dx * 64:(dx + 1) * 64],
                                    rhs, start=(dx == 0), stop=(dx == 4))
                        s = wpool.tile([64, 2 * NI2 * 8], fp16, tag="s2", name="s2")
                        psv = ps[:, :].rearrange("p (two n) -> p two n", two=2)
                        sv = s[:, :].rearrange("p (two n) -> p two n", two=2)
                        nc.vector.tensor_scalar(
                            sv[:, :, :NI2 * 8], psv[:, :, :NI2 * 8],
                            bnv[:64, 6:7], bnv[:64, 7:8], op0=OP.max, op1=OP.min)
                        svs.append(s)
                    for pair in range(2):
                        s = svs[pair]
                        sq = s[:, :].rearrange("p (sub i x2 xp) -> p sub i x2 xp",
                                               sub=2, i=NI2, xp=2)
                        psy = psB.tile([32, 2 * NI2 * 4], f32, tag="cpy", name="py2ps", bufs=1)
                        pyv = psy[:, :].rearrange("p (sub i x2) -> p sub i x2",
                                                  sub=2, i=NI2)
                        for xp in range(2):
                            nc.tensor.matmul(pyv, poolm2[:, :], sq[:, :, :, :, xp],
                                             start=(xp == 0), stop=(xp == 1))
                        nc.vector.tensor_scalar(
                            h2u_g[g][:, pair * 256:(pair + 1) * 256],
                            psy[:, :], bnv[:32, 8:9], None, op0=OP.is_ge)

                # FC stage
                rf = [fcpool.tile([128, NB], bf16, tag=f"rf{t}", name=f"rf{t}")
                      for t in range(2)]
                h2v = [h2u_g[g][:, :].rearrange("p (i x) -> p i x", x=4)
                       for g in range(2)]
                for t in range(2):
                    prf = psC.tile([128, NB], f32, tag="fcps", name="prf")
                    k = 0
                    for xl in range(2):
                        for g in range(2):
                            idx = (t * 2 + xl) * 2 + g
                            nc.tensor.matmul(
                                prf[:, :], selfm[:, idx * 128:(idx + 1) * 128],
                                h2v[g][:, :, t * 2 + xl],
                                start=(k == 0), stop=(k == 3))
                            k += 1
                    nc.scalar.activation(rf[t][:, :], prf[:, :], AF.Copy)
                psf = psC.tile([120, NB], f32, tag="fcps", name="psf")
                nc.tensor.matmul(psf[:, :], lhsTf1[:, 0:120], rf[0][:, :],
                                 start=True, stop=False)
                nc.tensor.matmul(psf[:, :], lhsTf1[:, 120:240], rf[1][:, :],
                                 start=False, stop=True)
                f1 = fcpool.tile([120, NB], bf16, tag="f1", name="f1")
                nc.scalar.activation(f1[:, :], psf[:, :], AF.Sign,
                                     bias=bnv[:120, 10:11], scale=bnv[:120, 9:10])
                psf2 = psC.tile([84, NB], f32, tag="fcps", name="psf2")
                nc.tensor.matmul(psf2[:, :], lhsTf2[:, :], f1[:, :],
                                 start=True, stop=True)
                f2 = fcpool.tile([84, NB], fp16, tag="f2", name="f2")
                nc.scalar.activation(f2[:, :], psf2[:, :], AF.Sign,
                                     bias=bnv[:84, 12:13], scale=bnv[:84, 11:12])
                # fused fc3+bn3+transpose: stationary = f2 data (84xNB),
                # moving = rhs3 (84x10) -> psum [NB, 10] already transposed;
                # bias via rank-1 accumulate (ones x shift).
                pso = psC.tile([NB, 10], f32, tag="fcps", name="pso")
                nc.tensor.matmul(pso[:, :], f2[:, :], rhs3[:, :],
                                 start=True, stop=False)
                nc.tensor.matmul(pso[:, :], ones1[:, :], rhs3s[:, :],
                                 start=False, stop=True)
                ot = fcpool.tile([NB, 10], f32, tag="ot", name="ot")
                nc.scalar.activation(ot[:, :], pso[:, :], AF.Copy)
                nc.sync.dma_start(out=out_t.ap()[blk * NB:(blk + 1) * NB, :],
                                  in_=ot[:, :])
                return h2u_g, f1, ot

            # two-stage software pipeline over blocks
            pending = None
            for blk in range(NBLK + 1):
                new_pending = None
                if blk < NBLK:
                    xr, h1u_c = stage_a(blk)
                    new_pending = (blk, h1u_c)
                if pending is not None:
                    stage_b(*pending)
                pending = new_pending
    legalize_waits(nc)
    return nc


def kernel(**inputs):
    inputs = {k: np.asarray(v) for k, v in inputs.items()}
    d = {k: v.astype(np.float32) if v.dtype != np.float32 else v
         for k, v in inputs.items()}
    C = build_constants(d)
    packed = pack_constants(C)

    if 'nc' not in _cache:
        _cache['nc'] = build_bass()
    nc = _cache['nc']

    x = d['x'].reshape(B, 784)
    in_maps = []
    for c in range(NCORES):
        m = {'x': np.ascontiguousarray(x[c * BPC:(c + 1) * BPC])}
        for k, v in packed.items():
            m[k] = np.ascontiguousarray(v.astype(np.float32))
        in_maps.append(m)

    import os
    from concourse import bass_utils
    trace = os.environ.get("BASS_TRACE", "0") == "1"
    res = bass_utils.run_bass_kernel_spmd(nc, in_maps, core_ids=list(range(NCORES)),
                                          trace=trace)
    _cache['last_results'] = res
    out = np.concatenate([r['out'] for r in res.results], axis=0)
    return out.astype(np.float32)

